# revision 23
# baseline (speedup 1.0000x reference)
"""Multi-head attention + layernorm Bass kernel for Trainium2, 8 cores.

Problem: B=8, S=1024, D=768, H=12 heads x DH=64, key-padding mask, softmax,
output projection, layernorm.  Sharding: pure data parallelism — one batch
element per NeuronCore, no collectives.

v3 layout (matmul operands fp16, accumulation fp32 in PSUM):
  - x^T loaded as per-chunk DMAs with the first pairs' weights ahead of the
    second x half, spread over both HWDGE queues, so projections start
    within a few us.
  - q/k projections as [128,512] half-tiles through a 1-bank PSUM ring.
  - attention per head-pair is ACT(exp)-bound: the j-loop is emitted
    software-pipelined (ctx matmuls delayed one chunk) and a global filler
    queue (later pairs' q/k projections, v projections, the pairs-0..4
    partial out-projection) breathes through the exp-wait bubbles so the
    PE never idles long enough for the HAM clock gate to re-throttle.
  - softmax denominators drop out of psum row 64 (ones-column in V); ctx
    is evacuated to SBUF f32, den rows batch at partitions 0/32 for one
    reciprocal_approx_fast, a rank-1 (K=1) matmul broadcasts 1/den, and
    the normalize multiply runs promptly so the psum ring never waits.
  - out projection: pairs 0-4 + bias accumulate into an SBUF fp16 partial
    during the last attention; the tail adds pair 5 plus the partial via
    an identity-weight matmul (no DVE adds), then layernorm via
    bn_stats/bn_aggr straight off PSUM with fp16 gamma/beta and output.
"""

import numpy as np

B, S, D, H, DH = 8, 1024, 768, 12, 64
NPAIR, NQUAD = H // 2, H // 4
SBLK = S // 128      # 8 key/row chunks
DCH = D // 128       # 6 contraction chunks
LN_EPS = 1e-5
NEG_MASK = -30.0

_PROGRAM = None


def _build_program():
    import concourse.bass as bass
    from concourse import bacc
    import concourse.tile as tile
    import concourse.mybir as mybir
    from contextlib import ExitStack

    F32 = mybir.dt.float32
    F32R = mybir.dt.float32r
    F16 = mybir.dt.float16
    AF = mybir.ActivationFunctionType

    nc = bacc.Bacc("TRN2", target_bir_lowering=False)

    xt_d = nc.dram_tensor("xt", [128, DCH * S], F16, kind="ExternalInput")
    wq_d = nc.dram_tensor("wq", [NPAIR, 128, DCH * 128], F16, kind="ExternalInput")
    wk_d = nc.dram_tensor("wk", [NPAIR, 128, DCH * 128], F16, kind="ExternalInput")
    wv_d = nc.dram_tensor("wv", [NQUAD, 128, DCH * 260], F16, kind="ExternalInput")
    wo_d = nc.dram_tensor("wo", [128, DCH * D], F16, kind="ExternalInput")
    bqk_d = nc.dram_tensor("bqk", [128, 2 * NPAIR], F32, kind="ExternalInput")
    bv_d = nc.dram_tensor("bv", [1, NQUAD * 260], F32, kind="ExternalInput")
    maskb_d = nc.dram_tensor("maskb", [128, SBLK], F32, kind="ExternalInput")
    gamma_d = nc.dram_tensor("gamma16", [1, D], F16, kind="ExternalInput")
    beta_d = nc.dram_tensor("beta16", [1, D], F16, kind="ExternalInput")
    onesr_d = nc.dram_tensor("onesr", [1, 128], F32R, kind="ExternalInput")
    bor_d = nc.dram_tensor("bor", [1, D], F32R, kind="ExternalInput")
    ident_d = nc.dram_tensor("ident", [128, 128], F16, kind="ExternalInput")
    out_d = nc.dram_tensor("out", [S, D], F16, kind="ExternalOutput")

    with tile.TileContext(nc) as tc, ExitStack() as ctx:
        const = ctx.enter_context(tc.tile_pool(name="const", bufs=1))
        xt_p = ctx.enter_context(tc.tile_pool(name="xt_p", bufs=1))
        w_p = ctx.enter_context(tc.tile_pool(name="w_p", bufs=1))
        qk_p = ctx.enter_context(tc.tile_pool(name="qk_p", bufs=1))
        v_p = ctx.enter_context(tc.tile_pool(name="v_p", bufs=1))
        e_p = ctx.enter_context(tc.tile_pool(name="e_p", bufs=1))
        cx_p = ctx.enter_context(tc.tile_pool(name="cx_p", bufs=1))
        z_p = ctx.enter_context(tc.tile_pool(name="z_p", bufs=1))
        po_p = ctx.enter_context(tc.tile_pool(name="po_p", bufs=1))
        # 8 PSUM banks: proj 2x[128,512] + scores 2x[128,1024] + cx 2x[128,512]
        ps = ctx.enter_context(tc.tile_pool(name="ps", bufs=1, space="PSUM"))

        # ---- x^T per-chunk, weights for early pairs ahead of second half ----
        xta = xt_p.tile([128, DCH, S], F16, name="xta")
        xt = [xta[:, c, :] for c in range(DCH)]

        def load_xt(c, q):
            q.dma_start(out=xta[:, c, :], in_=xt_d[:, c * S:(c + 1) * S])

        wq_ts, wk_ts, wv_ts = [None] * NPAIR, [None] * NPAIR, [None] * NQUAD

        def load_wqk(p, q):
            wqp = w_p.tile([128, DCH, 128], F16, name="wqp", bufs=NPAIR)
            q.dma_start(out=wqp, in_=wq_d[p])
            wq_ts[p] = [wqp[:, c, :] for c in range(DCH)]
            wkp = w_p.tile([128, DCH, 128], F16, name="wkp", bufs=NPAIR)
            q.dma_start(out=wkp, in_=wk_d[p])
            wk_ts[p] = [wkp[:, c, :] for c in range(DCH)]

        def load_wv(qd, q):
            wvq = w_p.tile([128, DCH, 260], F16, name="wvq", bufs=NQUAD)
            q.dma_start(out=wvq, in_=wv_d[qd])
            wv_ts[qd] = [wvq[:, c, :] for c in range(DCH)]

        # sync queue: x chunks 0-2 first, then consts + later-pair weights
        for c in range(3):
            load_xt(c, nc.sync)
        bqk_t = const.tile([128, 2 * NPAIR], F32)
        nc.sync.dma_start(out=bqk_t, in_=bqk_d[:, :])
        mask_t = const.tile([128, SBLK], F32)
        nc.sync.dma_start(out=mask_t, in_=maskb_d[:, :])
        bv_t = const.tile([128, NQUAD * 260], F32)
        nc.sync.dma_start(out=bv_t, in_=bv_d[0:1, :].to_broadcast([128, NQUAD * 260]))

        # scalar queue: first pairs' weights, then x chunks 3-5, then the rest
        load_wqk(0, nc.scalar)
        load_wv(0, nc.scalar)
        for c in range(3, DCH):
            load_xt(c, nc.scalar)
        load_wqk(1, nc.scalar)
        load_wqk(2, nc.scalar)

        # remaining loads balanced across both queues
        load_wv(1, nc.sync)
        load_wqk(3, nc.sync)
        load_wv(2, nc.sync)
        load_wqk(4, nc.sync)
        load_wqk(5, nc.sync)
        woa = w_p.tile([128, DCH, D], F16, name="woa", bufs=1)
        nc.scalar.dma_start(out=woa, in_=wo_d[:, :])
        wo_t = [woa[:, c, :] for c in range(DCH)]
        onesr_t = const.tile([1, 128], F32R)
        nc.sync.dma_start(out=onesr_t, in_=onesr_d[:, :])
        bor_t = const.tile([1, D], F32R)
        nc.sync.dma_start(out=bor_t, in_=bor_d[:, :])
        ident_t = const.tile([128, 128], F16)
        nc.sync.dma_start(out=ident_t, in_=ident_d[:, :])
        gamma_t = const.tile([128, D], F16)
        nc.sync.dma_start(out=gamma_t, in_=gamma_d[0:1, :].to_broadcast([128, D]))
        beta_t = const.tile([128, D], F16)
        nc.sync.dma_start(out=beta_t, in_=beta_d[0:1, :].to_broadcast([128, D]))
        eps_t = const.tile([128, 1], F32)
        nc.vector.memset(eps_t, LN_EPS)
        onesc_t = const.tile([33, 64], F32R)
        nc.vector.memset(onesc_t.bitcast(F32), 1.0)

        from concourse.dve_ops import (
            RECIP_APPROX_FAST_CONSTS as _RC,
            RECIPROCAL_APPROX_FAST as _RAF,
        )

        def recip_approx_f32r(out, in_):
            # reciprocal_approx_fast with an f32r destination (same bits;
            # the public wrapper only accepts f32 outputs)
            return nc.vector._custom_dve(
                _RAF, out=out, in0=in_,
                s0=_RC["s0"], s1=_RC["s1"], imm2=_RC["imm2"])

        # ---- persistent SBUF results ----
        qt_of, kt_of = {}, {}           # pair -> [128, S] f16
        ctxt = []                       # pair -> [128, S] f16 normalized ctx^T
        for p in range(NPAIR):
            ctxt.append(cx_p.tile([128, S], F16, name="ctxt", bufs=NPAIR))
        po_acc = po_p.tile([128, SBLK, D], F16, name="po_acc", bufs=1)

        # ================= filler closure factories =================
        def qk_closures(p):
            """q/k projection of pair p as a list of small emit-closures."""
            cls = []
            qt = qk_p.tile([128, S], F16, name="qt_sb", bufs=3)
            kt = qk_p.tile([128, S], F16, name="kt_sb", bufs=3)
            qt_of[p], kt_of[p] = qt, kt
            for dst, wt, bcol in ((qt, wq_ts, p), (kt, wk_ts, NPAIR + p)):
                for half in range(2):
                    state = {}

                    def c1(wt=wt, p=p, half=half, state=state):
                        ph = ps.tile([128, 512], F32, name="psqk", tag="proj",
                                     bufs=2)
                        state["ph"] = ph
                        for c in range(3):
                            nc.tensor.matmul(
                                ph, wt[p][c],
                                xt[c][:, half * 512:(half + 1) * 512],
                                start=(c == 0), stop=False)

                    def c2(dst=dst, p=p, half=half, bcol=bcol, wt=wt,
                           state=state):
                        ph = state["ph"]
                        for c in range(3, DCH):
                            nc.tensor.matmul(
                                ph, wt[p][c],
                                xt[c][:, half * 512:(half + 1) * 512],
                                start=False, stop=(c == DCH - 1))
                        with tc.high_priority(offset=400):
                            nc.vector.tensor_scalar_add(
                                out=dst[:, half * 512:(half + 1) * 512],
                                in0=ph, scalar1=bqk_t[:, bcol:bcol + 1])

                    cls += [c1, c2]
            return cls

        v_sb = {}   # (quad, sblk) -> [128, 260] f16

        def vq_closures(qd):
            """v projection of quad qd as emit-closures (2 per seq block)."""
            cls = []
            for s in range(SBLK):
                state = {}

                def c1(qd=qd, s=s, state=state):
                    psv = ps.tile([128, 260], F32, name="psv", tag="proj",
                                  bufs=2, padded_shape=[128, 512])
                    state["psv"] = psv
                    for c in range(3):
                        nc.tensor.matmul(
                            psv, xt[c][:, s * 128:(s + 1) * 128], wv_ts[qd][c],
                            start=(c == 0), stop=False)

                def c2(qd=qd, s=s, state=state):
                    psv = state["psv"]
                    for c in range(3, DCH):
                        nc.tensor.matmul(
                            psv, xt[c][:, s * 128:(s + 1) * 128], wv_ts[qd][c],
                            start=False, stop=(c == DCH - 1))
                    vt = v_p.tile([128, 260], F16, name="v_sb", bufs=3 * SBLK)
                    nc.vector.tensor_add(out=vt, in0=psv,
                                         in1=bv_t[:, qd * 260:(qd + 1) * 260])
                    v_sb[(qd, s)] = vt

                cls += [c1, c2]
            return cls

        def partial_outproj_closures():
            """pairs 0-3 + bo of the out projection -> po_acc (fp16)."""
            cls = []
            for s in range(SBLK):
                state = {}

                def c1(s=s, state=state):
                    pa = ps.tile([128, 512], F32, name="pso_a", tag="proj",
                                 bufs=2)
                    state["pa"] = pa
                    for p in range(2):
                        nc.tensor.matmul(pa, ctxt[p][:, s * 128:(s + 1) * 128],
                                         wo_t[p][:, 0:512],
                                         start=(p == 0), stop=False)

                def c2(s=s, state=state):
                    pa = state["pa"]
                    for p in range(2, 4):
                        nc.tensor.matmul(pa, ctxt[p][:, s * 128:(s + 1) * 128],
                                         wo_t[p][:, 0:512],
                                         start=False, stop=False)

                def c2b(s=s, state=state):
                    pa = state["pa"]
                    nc.tensor.matmul(pa, onesr_t, bor_t[:, 0:512],
                                     start=False, stop=True)
                    pb = ps.tile([128, 256], F32, name="pso_b", tag="proj",
                                 bufs=2, padded_shape=[128, 512])
                    state["pb"] = pb
                    nc.tensor.matmul(pb, ctxt[0][:, s * 128:(s + 1) * 128],
                                     wo_t[0][:, 512:768],
                                     start=True, stop=False)

                def c3(s=s, state=state):
                    pb = state["pb"]
                    for p in range(1, 4):
                        nc.tensor.matmul(pb, ctxt[p][:, s * 128:(s + 1) * 128],
                                         wo_t[p][:, 512:768],
                                         start=False, stop=False)
                    nc.tensor.matmul(pb, onesr_t, bor_t[:, 512:768],
                                     start=False, stop=True)

                def c4(s=s, state=state):
                    nc.vector.tensor_copy(out=po_acc[:, s, 0:512],
                                          in_=state["pa"])
                    nc.vector.tensor_copy(out=po_acc[:, s, 512:768],
                                          in_=state["pb"])

                cls += [c1, c2, c2b, c3, c4]
            return cls

        # ================= attention =================
        def attention(p, fq):
            qt, kt = qt_of[p], kt_of[p]
            ct = ctxt[p]
            qd, l0 = divmod(2 * p, 4)
            for iblk in range(2):
                pcx = [ps.tile([128, 512], F32, name="pscx", tag="cx", bufs=2)
                       for _ in range(2)]
                pend = None
                for j in range(SBLK):
                    pst = ps.tile([128, 1024], F32, name="psst", tag="st",
                                  bufs=2)
                    nc.tensor.matmul(
                        pst[:, 0:512], kt[0:64, j * 128:(j + 1) * 128],
                        qt[0:64, iblk * 512:(iblk + 1) * 512],
                        start=True, stop=True, tile_position=(0, 0))
                    nc.tensor.matmul(
                        pst[:, 512:1024], kt[64:128, j * 128:(j + 1) * 128],
                        qt[64:128, iblk * 512:(iblk + 1) * 512],
                        start=True, stop=True, tile_position=(64, 0))
                    et = e_p.tile([128, 1024], F16, name="expt", bufs=4)
                    nc.scalar.activation(et, pst, AF.Exp, bias=mask_t[:, j:j + 1])
                    if fq:
                        fq.pop(0)()
                    if pend is not None:
                        pend()
                    def pend(j=j, et=et, pcx=pcx, qd=qd, l0=l0):
                        for idx in range(2):
                            vsl = v_sb[(qd, j)][:, (l0 + idx) * 65:
                                                (l0 + idx + 1) * 65]
                            nc.tensor.matmul(pcx[idx][0:65, :], vsl,
                                             et[:, idx * 512:(idx + 1) * 512],
                                             start=(j == 0),
                                             stop=(j == SBLK - 1))
                pend()
                # evacuate ctx to SBUF f32 (frees the psum bank); batch the
                # two den rows at partitions 0/32 (custom-DVE ops and matmul
                # operands need 0/32/64 bases), one approx-reciprocal, then
                # rank-1 broadcast + prompt normalize so the psum ring and
                # the out-projection never wait long
                stages = []
                dd = z_p.tile([33, 512], F32, name="den_rows", bufs=3)
                for idx in range(2):
                    stage = v_p.tile([64, 512], F32, name="cx_stage", bufs=4)
                    nc.vector.tensor_copy(out=stage, in_=pcx[idx][0:64, :])
                    stages.append(stage)
                    nc.vector.tensor_copy(out=dd[32 * idx:32 * idx + 1, :],
                                          in_=pcx[idx][64:65, :])
                ddinv = z_p.tile([33, 512], F32R, name="rinv_rows", bufs=3)
                recip_approx_f32r(out=ddinv, in_=dd)
                for idx in range(2):
                    pbc = ps.tile([64, 512], F32, name="psbc", tag="cx",
                                  bufs=2, padded_shape=[128, 512])
                    nc.tensor.matmul(
                        pbc, onesc_t[32 * idx:32 * idx + 1, :],
                        ddinv[32 * idx:32 * idx + 1, :],
                        start=True, stop=True)
                    csl = ct[idx * 64:(idx + 1) * 64,
                             iblk * 512:(iblk + 1) * 512]
                    nc.vector.tensor_mul(out=csl, in0=stages[idx], in1=pbc)
            # flush leftovers: every filler must be emitted before the next
            # attention reads what it writes (deps follow emission order)
            while fq:
                fq.pop(0)()

        # ================= schedule =================
        for cl in qk_closures(0) + vq_closures(0) + qk_closures(1):
            cl()
        attention(0, vq_closures(1))
        attention(1, qk_closures(2))
        vq2 = vq_closures(2)
        attention(2, qk_closures(3) + vq2[:8])
        attention(3, vq2[8:] + qk_closures(4))
        po_cls = partial_outproj_closures()
        attention(4, qk_closures(5) + po_cls[:8])
        attention(5, po_cls[8:24])
        for cl in po_cls[24:]:
            cl()

        # ---- tail: pair 5 + partial via identity matmul + layernorm ----
        for s in range(SBLK):
            pa = ps.tile([128, 512], F32, name="ps2a", tag="proj", bufs=2)
            for p in (4, 5):
                nc.tensor.matmul(pa, ctxt[p][:, s * 128:(s + 1) * 128],
                                 wo_t[p][:, 0:512],
                                 start=(p == 4), stop=False)
            nc.tensor.matmul(pa, ident_t, po_acc[:, s, 0:512],
                             start=False, stop=True)
            pb = ps.tile([128, 256], F32, name="ps2b", tag="proj", bufs=2,
                         padded_shape=[128, 512])
            for p in (4, 5):
                nc.tensor.matmul(pb, ctxt[p][:, s * 128:(s + 1) * 128],
                                 wo_t[p][:, 512:768],
                                 start=(p == 4), stop=False)
            nc.tensor.matmul(pb, ident_t, po_acc[:, s, 512:768],
                             start=False, stop=True)
            stats = z_p.tile([128, 3, 6], F32, name="stats", bufs=2)
            for g in range(2):
                nc.vector.bn_stats(out=stats[:, g, :],
                                   in_=pa[:, g * 256:(g + 1) * 256])
            nc.vector.bn_stats(out=stats[:, 2, :], in_=pb)
            mv = z_p.tile([128, 2], F32, name="mv", bufs=2)
            nc.vector.bn_aggr(out=mv, in_=stats)
            stdv = z_p.tile([128, 1], F32, name="stdv", bufs=2)
            nc.scalar.activation(stdv, mv[:, 1:2], AF.Sqrt, bias=eps_t)
            rstd = z_p.tile([128, 1], F32, name="rstd", bufs=2)
            nc.vector.reciprocal(out=rstd, in_=stdv)
            nmr = z_p.tile([128, 1], F32, name="nmr", bufs=2)
            nc.vector.tensor_scalar(out=nmr, in0=mv[:, 0:1], scalar1=rstd,
                                    scalar2=-1.0, op0=mybir.AluOpType.mult,
                                    op1=mybir.AluOpType.mult)
            z = z_p.tile([128, D], F16, name="z_sb", bufs=2)
            nc.scalar.activation(z[:, 0:512], pa, AF.Identity, bias=nmr,
                                 scale=rstd)
            nc.scalar.activation(z[:, 512:768], pb, AF.Identity, bias=nmr,
                                 scale=rstd)
            nc.vector.tensor_mul(out=z, in0=z, in1=gamma_t)
            nc.vector.tensor_add(out=z, in0=z, in1=beta_t)
            nc.sync.dma_start(out=out_d[s * 128:(s + 1) * 128, :], in_=z)

    nc.compile()
    return nc


def _host_inputs(inputs):
    x = np.asarray(inputs["input_tensor"], np.float32)
    mask = np.asarray(inputs["attention_mask"])
    Wq = np.asarray(inputs["Wq"], np.float32)
    bq = np.asarray(inputs["bq"], np.float32)
    Wk = np.asarray(inputs["Wk"], np.float32)
    bk = np.asarray(inputs["bk"], np.float32)
    Wv = np.asarray(inputs["Wv"], np.float32)
    bv = np.asarray(inputs["bv"], np.float32)
    Wo = np.asarray(inputs["Wo"], np.float32)
    bo = np.asarray(inputs["bo"], np.float32)
    gamma = np.asarray(inputs["gamma"], np.float32)
    beta = np.asarray(inputs["beta"], np.float32)

    scale = np.float32(1.0 / np.sqrt(DH))
    wq_flat = np.ascontiguousarray(
        (Wq * scale).transpose(1, 0, 2).reshape(D, D))
    wk_flat = np.ascontiguousarray(Wk.transpose(1, 0, 2).reshape(D, D))
    bq_s = (bq * scale).reshape(D)
    bk_s = bk.reshape(D)

    wv_aug = np.zeros((D, NQUAD * 260), np.float32)
    bv_aug = np.zeros((1, NQUAD * 260), np.float32)
    for h in range(H):
        q, l = divmod(h, 4)
        base = q * 260 + l * 65
        wv_aug[:, base:base + 64] = Wv[h]
        bv_aug[0, base:base + 64] = bv[h]
        bv_aug[0, base + 64] = 1.0

    bqk = np.zeros((128, 2 * NPAIR), np.float32)
    for p in range(NPAIR):
        bqk[:, p] = bq_s[p * 128:(p + 1) * 128]
        bqk[:, NPAIR + p] = bk_s[p * 128:(p + 1) * 128]

    def sbuf_layout(w, width):
        # [D, n*width] -> [n, 128, DCH*width]: partition-major per tile
        n = w.shape[1] // width
        return np.ascontiguousarray(
            w.reshape(DCH, 128, n, width).transpose(2, 1, 0, 3).reshape(
                n, 128, DCH * width).astype(np.float16))

    shared = {
        "wq": sbuf_layout(wq_flat, 128), "wk": sbuf_layout(wk_flat, 128),
        "wv": sbuf_layout(wv_aug, 260),
        "wo": sbuf_layout(np.ascontiguousarray(Wo), D)[0],
        "bqk": bqk, "bv": bv_aug,
        "gamma16": gamma.reshape(1, D).astype(np.float16),
        "beta16": beta.reshape(1, D).astype(np.float16),
        "onesr": np.ones((1, 128), np.float32),
        "bor": bo.reshape(1, D).copy(),
        "ident": np.eye(128, dtype=np.float16),
    }
    in_maps = []
    for b in range(B):
        mb = np.where(mask[b], 0.0, NEG_MASK).astype(np.float32)
        in_maps.append({
            **shared,
            "xt": np.ascontiguousarray(
                x[b].T.reshape(DCH, 128, S).transpose(1, 0, 2).reshape(
                    128, DCH * S).astype(np.float16)),
            "maskb": np.ascontiguousarray(mb.reshape(SBLK, 128).T),
        })
    return in_maps


def _get_program():
    global _PROGRAM
    if _PROGRAM is None:
        _PROGRAM = _build_program()
    return _PROGRAM


def kernel(**inputs):
    from concourse.bass_utils import run_bass_kernel_spmd

    nc = _get_program()
    in_maps = _host_inputs(inputs)
    res = run_bass_kernel_spmd(nc, in_maps, list(range(B)))
    return np.stack(
        [res.results[b]["out"].astype(np.float32) for b in range(B)], axis=0)


if __name__ == "__main__":
    rng = np.random.default_rng(0)
    demo = {
        "input_tensor": rng.standard_normal((B, S, D)).astype(np.float32),
        "attention_mask": np.ones((B, S), bool),
        "Wq": rng.standard_normal((H, D, DH)).astype(np.float32) * 0.03,
        "bq": rng.standard_normal((H, DH)).astype(np.float32) * 0.03,
        "Wk": rng.standard_normal((H, D, DH)).astype(np.float32) * 0.03,
        "bk": rng.standard_normal((H, DH)).astype(np.float32) * 0.03,
        "Wv": rng.standard_normal((H, D, DH)).astype(np.float32) * 0.03,
        "bv": rng.standard_normal((H, DH)).astype(np.float32) * 0.03,
        "Wo": rng.standard_normal((D, D)).astype(np.float32) * 0.03,
        "bo": rng.standard_normal((D,)).astype(np.float32) * 0.03,
        "gamma": np.ones((D,), np.float32),
        "beta": np.zeros((D,), np.float32),
    }
    out = kernel(**demo)
    print("kernel ran, out shape", out.shape, "finite:", np.isfinite(out).all())


# revision 24
# speedup vs baseline: 1.2660x; 1.2660x over previous
"""Multi-head attention + layernorm Bass kernel for Trainium2, 8 cores.

Problem: B=8, S=1024, D=768, H=12 heads x DH=64, key-padding mask, softmax,
output projection, layernorm.  Sharding: pure data parallelism — one batch
element per NeuronCore, no collectives.

v3 layout (matmul operands fp16, accumulation fp32 in PSUM):
  - x^T loaded as per-chunk DMAs with the first pairs' weights ahead of the
    second x half, spread over both HWDGE queues, so projections start
    within a few us.
  - q/k projections as [128,512] half-tiles through a 1-bank PSUM ring.
  - attention per head-pair is ACT(exp)-bound: the j-loop is emitted
    software-pipelined (ctx matmuls delayed one chunk) and a global filler
    queue (later pairs' q/k projections, v projections, the pairs-0..4
    partial out-projection) breathes through the exp-wait bubbles so the
    PE never idles long enough for the HAM clock gate to re-throttle.
  - softmax denominators drop out of psum row 64 (ones-column in V); ctx
    is evacuated to SBUF f32, den rows batch at partitions 0/32 for one
    reciprocal_approx_fast, a rank-1 (K=1) matmul broadcasts 1/den, and
    the normalize multiply runs promptly so the psum ring never waits.
  - out projection: pairs 0-4 + bias accumulate into an SBUF fp16 partial
    during the last attention; the tail adds pair 5 plus the partial via
    an identity-weight matmul (no DVE adds), then layernorm via
    bn_stats/bn_aggr straight off PSUM with fp16 gamma/beta and output.
"""

import numpy as np

B, S, D, H, DH = 8, 1024, 768, 12, 64
NPAIR, NQUAD = H // 2, H // 4
SBLK = S // 128      # 8 key/row chunks
DCH = D // 128       # 6 contraction chunks
LN_EPS = 1e-5
NEG_MASK = -30.0

_PROGRAM = None


def _build_program():
    import concourse.bass as bass
    from concourse import bacc
    import concourse.tile as tile
    import concourse.mybir as mybir
    from contextlib import ExitStack

    F32 = mybir.dt.float32
    F32R = mybir.dt.float32r
    F16 = mybir.dt.float16
    AF = mybir.ActivationFunctionType

    nc = bacc.Bacc("TRN2", target_bir_lowering=False)

    xt_d = nc.dram_tensor("xt", [128, DCH * S], F16, kind="ExternalInput")
    wq_d = nc.dram_tensor("wq", [NPAIR, 128, DCH * 128], F16, kind="ExternalInput")
    wk_d = nc.dram_tensor("wk", [NPAIR, 128, DCH * 128], F16, kind="ExternalInput")
    wv_d = nc.dram_tensor("wv", [NQUAD, 128, DCH * 260], F16, kind="ExternalInput")
    wo_d = nc.dram_tensor("wo", [128, DCH * D], F16, kind="ExternalInput")
    bqk_d = nc.dram_tensor("bqk", [128, 2 * NPAIR], F32, kind="ExternalInput")
    bv_d = nc.dram_tensor("bv", [1, NQUAD * 260], F32, kind="ExternalInput")
    maskb_d = nc.dram_tensor("maskb", [128, SBLK], F32, kind="ExternalInput")
    gamma_d = nc.dram_tensor("gamma16", [1, D], F16, kind="ExternalInput")
    beta_d = nc.dram_tensor("beta16", [1, D], F16, kind="ExternalInput")
    onesr_d = nc.dram_tensor("onesr", [1, 128], F32R, kind="ExternalInput")
    bor_d = nc.dram_tensor("bor", [1, D], F32R, kind="ExternalInput")
    ident_d = nc.dram_tensor("ident", [128, 128], F16, kind="ExternalInput")
    out_d = nc.dram_tensor("out", [S, D], F16, kind="ExternalOutput")

    with tile.TileContext(nc) as tc, ExitStack() as ctx:
        const = ctx.enter_context(tc.tile_pool(name="const", bufs=1))
        xt_p = ctx.enter_context(tc.tile_pool(name="xt_p", bufs=1))
        w_p = ctx.enter_context(tc.tile_pool(name="w_p", bufs=1))
        qk_p = ctx.enter_context(tc.tile_pool(name="qk_p", bufs=1))
        v_p = ctx.enter_context(tc.tile_pool(name="v_p", bufs=1))
        e_p = ctx.enter_context(tc.tile_pool(name="e_p", bufs=1))
        cx_p = ctx.enter_context(tc.tile_pool(name="cx_p", bufs=1))
        z_p = ctx.enter_context(tc.tile_pool(name="z_p", bufs=1))
        po_p = ctx.enter_context(tc.tile_pool(name="po_p", bufs=1))
        # 8 PSUM banks: proj 2x[128,512] + scores 2x[128,1024] + cx 2x[128,512]
        ps = ctx.enter_context(tc.tile_pool(name="ps", bufs=1, space="PSUM"))

        # ---- x^T per-chunk, weights for early pairs ahead of second half ----
        xta = xt_p.tile([128, DCH, S], F16, name="xta")
        xt = [xta[:, c, :] for c in range(DCH)]

        def load_xt(c, q):
            q.dma_start(out=xta[:, c, :], in_=xt_d[:, c * S:(c + 1) * S])

        wq_ts, wk_ts, wv_ts = [None] * NPAIR, [None] * NPAIR, [None] * NQUAD

        def load_wqk(p, q):
            wqp = w_p.tile([128, DCH, 128], F16, name="wqp", bufs=NPAIR)
            q.dma_start(out=wqp, in_=wq_d[p])
            wq_ts[p] = [wqp[:, c, :] for c in range(DCH)]
            wkp = w_p.tile([128, DCH, 128], F16, name="wkp", bufs=NPAIR)
            q.dma_start(out=wkp, in_=wk_d[p])
            wk_ts[p] = [wkp[:, c, :] for c in range(DCH)]

        def load_wv(qd, q):
            wvq = w_p.tile([128, DCH, 260], F16, name="wvq", bufs=NQUAD)
            q.dma_start(out=wvq, in_=wv_d[qd])
            wv_ts[qd] = [wvq[:, c, :] for c in range(DCH)]

        # sync queue: x chunks 0-2 first, then consts + later-pair weights
        for c in range(3):
            load_xt(c, nc.sync)
        bqk_t = const.tile([128, 2 * NPAIR], F32)
        nc.sync.dma_start(out=bqk_t, in_=bqk_d[:, :])
        mask_t = const.tile([128, SBLK], F32)
        nc.sync.dma_start(out=mask_t, in_=maskb_d[:, :])
        bv_t = const.tile([128, NQUAD * 260], F32)
        nc.sync.dma_start(out=bv_t, in_=bv_d[0:1, :].to_broadcast([128, NQUAD * 260]))

        # scalar queue: first pairs' weights, then x chunks 3-5, then the rest
        load_wqk(0, nc.scalar)
        load_wv(0, nc.scalar)
        for c in range(3, DCH):
            load_xt(c, nc.scalar)
        load_wqk(1, nc.scalar)
        load_wqk(2, nc.scalar)

        # remaining loads balanced across both queues
        load_wv(1, nc.sync)
        load_wqk(3, nc.sync)
        load_wv(2, nc.sync)
        load_wqk(4, nc.sync)
        load_wqk(5, nc.sync)
        woa = w_p.tile([128, DCH, D], F16, name="woa", bufs=1)
        nc.scalar.dma_start(out=woa, in_=wo_d[:, :])
        wo_t = [woa[:, c, :] for c in range(DCH)]
        onesr_t = const.tile([1, 128], F32R)
        nc.sync.dma_start(out=onesr_t, in_=onesr_d[:, :])
        bor_t = const.tile([1, D], F32R)
        nc.sync.dma_start(out=bor_t, in_=bor_d[:, :])
        ident_t = const.tile([128, 128], F16)
        nc.sync.dma_start(out=ident_t, in_=ident_d[:, :])
        gamma_t = const.tile([128, D], F16)
        nc.sync.dma_start(out=gamma_t, in_=gamma_d[0:1, :].to_broadcast([128, D]))
        beta_t = const.tile([128, D], F16)
        nc.sync.dma_start(out=beta_t, in_=beta_d[0:1, :].to_broadcast([128, D]))
        eps_t = const.tile([128, 1], F32)
        nc.vector.memset(eps_t, LN_EPS)
        onesc_t = const.tile([33, 64], F32R)
        nc.vector.memset(onesc_t.bitcast(F32), 1.0)

        from concourse.dve_ops import (
            RECIP_APPROX_FAST_CONSTS as _RC,
            RECIPROCAL_APPROX_FAST as _RAF,
        )

        def recip_approx_f32r(out, in_):
            # reciprocal_approx_fast with an f32r destination (same bits;
            # the public wrapper only accepts f32 outputs)
            return nc.vector._custom_dve(
                _RAF, out=out, in0=in_,
                s0=_RC["s0"], s1=_RC["s1"], imm2=_RC["imm2"])

        # ---- persistent SBUF results ----
        qt_of, kt_of = {}, {}           # pair -> [128, S] f16
        ctxt = []                       # pair -> [128, S] f16 normalized ctx^T
        for p in range(NPAIR):
            ctxt.append(cx_p.tile([128, S], F16, name="ctxt", bufs=NPAIR))
        po_acc = po_p.tile([128, SBLK, D], F16, name="po_acc", bufs=1)

        # ================= filler closure factories =================
        def qk_closures(p):
            """q/k projection of pair p as a list of small emit-closures."""
            cls = []
            qt = qk_p.tile([128, S], F16, name="qt_sb", bufs=3)
            kt = qk_p.tile([128, S], F16, name="kt_sb", bufs=3)
            qt_of[p], kt_of[p] = qt, kt
            for dst, wt, bcol in ((qt, wq_ts, p), (kt, wk_ts, NPAIR + p)):
                for half in range(2):
                    state = {}

                    def c1(wt=wt, p=p, half=half, state=state):
                        ph = ps.tile([128, 512], F32, name="psqk", tag="proj",
                                     bufs=2)
                        state["ph"] = ph
                        for c in range(3):
                            nc.tensor.matmul(
                                ph, wt[p][c],
                                xt[c][:, half * 512:(half + 1) * 512],
                                start=(c == 0), stop=False)

                    def c2(dst=dst, p=p, half=half, bcol=bcol, wt=wt,
                           state=state):
                        ph = state["ph"]
                        for c in range(3, DCH):
                            nc.tensor.matmul(
                                ph, wt[p][c],
                                xt[c][:, half * 512:(half + 1) * 512],
                                start=False, stop=(c == DCH - 1))
                        with tc.high_priority(offset=400):
                            nc.vector.tensor_scalar_add(
                                out=dst[:, half * 512:(half + 1) * 512],
                                in0=ph, scalar1=bqk_t[:, bcol:bcol + 1])

                    cls += [c1, c2]
            return cls

        v_sb = {}   # (quad, sblk) -> [128, 260] f16

        def vq_closures(qd):
            """v projection of quad qd as emit-closures (2 per seq block)."""
            cls = []
            for s in range(SBLK):
                state = {}

                def c1(qd=qd, s=s, state=state):
                    psv = ps.tile([128, 260], F32, name="psv", tag="proj",
                                  bufs=2, padded_shape=[128, 512])
                    state["psv"] = psv
                    for c in range(3):
                        nc.tensor.matmul(
                            psv, xt[c][:, s * 128:(s + 1) * 128], wv_ts[qd][c],
                            start=(c == 0), stop=False)

                def c2(qd=qd, s=s, state=state):
                    psv = state["psv"]
                    for c in range(3, DCH):
                        nc.tensor.matmul(
                            psv, xt[c][:, s * 128:(s + 1) * 128], wv_ts[qd][c],
                            start=False, stop=(c == DCH - 1))
                    vt = v_p.tile([128, 260], F16, name="v_sb", bufs=3 * SBLK)
                    nc.vector.tensor_add(out=vt, in0=psv,
                                         in1=bv_t[:, qd * 260:(qd + 1) * 260])
                    v_sb[(qd, s)] = vt

                cls += [c1, c2]
            return cls

        def partial_outproj_closures():
            """pairs 0-3 + bo of the out projection -> po_acc (fp16)."""
            cls = []
            for s in range(SBLK):
                state = {}

                def c1(s=s, state=state):
                    pa = ps.tile([128, 512], F32, name="pso_a", tag="proj",
                                 bufs=2)
                    state["pa"] = pa
                    for p in range(2):
                        nc.tensor.matmul(pa, ctxt[p][:, s * 128:(s + 1) * 128],
                                         wo_t[p][:, 0:512],
                                         start=(p == 0), stop=False)

                def c2(s=s, state=state):
                    pa = state["pa"]
                    for p in range(2, 4):
                        nc.tensor.matmul(pa, ctxt[p][:, s * 128:(s + 1) * 128],
                                         wo_t[p][:, 0:512],
                                         start=False, stop=False)

                def c2b(s=s, state=state):
                    pa = state["pa"]
                    nc.tensor.matmul(pa, onesr_t, bor_t[:, 0:512],
                                     start=False, stop=True)
                    pb = ps.tile([128, 256], F32, name="pso_b", tag="proj",
                                 bufs=2, padded_shape=[128, 512])
                    state["pb"] = pb
                    nc.tensor.matmul(pb, ctxt[0][:, s * 128:(s + 1) * 128],
                                     wo_t[0][:, 512:768],
                                     start=True, stop=False)

                def c3(s=s, state=state):
                    pb = state["pb"]
                    for p in range(1, 4):
                        nc.tensor.matmul(pb, ctxt[p][:, s * 128:(s + 1) * 128],
                                         wo_t[p][:, 512:768],
                                         start=False, stop=False)
                    nc.tensor.matmul(pb, onesr_t, bor_t[:, 512:768],
                                     start=False, stop=True)

                def c4(s=s, state=state):
                    nc.vector.tensor_copy(out=po_acc[:, s, 0:512],
                                          in_=state["pa"])
                    nc.vector.tensor_copy(out=po_acc[:, s, 512:768],
                                          in_=state["pb"])

                cls += [c1, c2, c2b, c3, c4]
            return cls

        # ================= attention =================
        def attention(p, fq):
            qt, kt = qt_of[p], kt_of[p]
            ct = ctxt[p]
            qd, l0 = divmod(2 * p, 4)
            for iblk in range(2):
                pcx = [ps.tile([128, 512], F32, name="pscx", tag="cx", bufs=2)
                       for _ in range(2)]
                pend = None
                for j in range(SBLK):
                    pst = ps.tile([128, 1024], F32, name="psst", tag="st",
                                  bufs=2)
                    nc.tensor.matmul(
                        pst[:, 0:512], kt[0:64, j * 128:(j + 1) * 128],
                        qt[0:64, iblk * 512:(iblk + 1) * 512],
                        start=True, stop=True, tile_position=(0, 0))
                    nc.tensor.matmul(
                        pst[:, 512:1024], kt[64:128, j * 128:(j + 1) * 128],
                        qt[64:128, iblk * 512:(iblk + 1) * 512],
                        start=True, stop=True, tile_position=(64, 0))
                    et = e_p.tile([128, 1024], F16, name="expt", bufs=4)
                    nc.scalar.activation(et, pst, AF.Exp, bias=mask_t[:, j:j + 1])
                    for _ in range(2):
                        if fq:
                            fq.pop(0)()
                    if pend is not None:
                        pend()
                    def pend(j=j, et=et, pcx=pcx, qd=qd, l0=l0):
                        for idx in range(2):
                            vsl = v_sb[(qd, j)][:, (l0 + idx) * 65:
                                                (l0 + idx + 1) * 65]
                            nc.tensor.matmul(pcx[idx][0:65, :], vsl,
                                             et[:, idx * 512:(idx + 1) * 512],
                                             start=(j == 0),
                                             stop=(j == SBLK - 1))
                pend()
                # evacuate ctx to SBUF f32 (frees the psum bank); batch the
                # two den rows at partitions 0/32 (custom-DVE ops and matmul
                # operands need 0/32/64 bases), one approx-reciprocal, then
                # rank-1 broadcast + prompt normalize so the psum ring and
                # the out-projection never wait long
                stages = []
                dd = z_p.tile([33, 512], F32, name="den_rows", bufs=3)
                for idx in range(2):
                    stage = v_p.tile([64, 512], F32, name="cx_stage", bufs=4)
                    nc.vector.tensor_copy(out=stage, in_=pcx[idx][0:64, :])
                    stages.append(stage)
                    nc.vector.tensor_copy(out=dd[32 * idx:32 * idx + 1, :],
                                          in_=pcx[idx][64:65, :])
                ddinv = z_p.tile([33, 512], F32R, name="rinv_rows", bufs=3)
                recip_approx_f32r(out=ddinv, in_=dd)
                for idx in range(2):
                    pbc = ps.tile([64, 512], F32, name="psbc", tag="cx",
                                  bufs=2, padded_shape=[128, 512])
                    nc.tensor.matmul(
                        pbc, onesc_t[32 * idx:32 * idx + 1, :],
                        ddinv[32 * idx:32 * idx + 1, :],
                        start=True, stop=True)
                    csl = ct[idx * 64:(idx + 1) * 64,
                             iblk * 512:(iblk + 1) * 512]
                    nc.vector.tensor_mul(out=csl, in0=stages[idx], in1=pbc)
            # flush leftovers: every filler must be emitted before the next
            # attention reads what it writes (deps follow emission order)
            while fq:
                fq.pop(0)()

        # ================= schedule =================
        for cl in qk_closures(0) + vq_closures(0) + qk_closures(1):
            cl()
        attention(0, vq_closures(1))
        attention(1, qk_closures(2))
        vq2 = vq_closures(2)
        attention(2, qk_closures(3) + vq2[:8])
        attention(3, vq2[8:] + qk_closures(4))
        po_cls = partial_outproj_closures()
        attention(4, qk_closures(5) + po_cls[:8])
        attention(5, po_cls[8:24])
        for cl in po_cls[24:]:
            cl()

        # ---- tail: pair 5 + partial via identity matmul + layernorm ----
        for s in range(SBLK):
            pa = ps.tile([128, 512], F32, name="ps2a", tag="proj", bufs=2)
            for p in (4, 5):
                nc.tensor.matmul(pa, ctxt[p][:, s * 128:(s + 1) * 128],
                                 wo_t[p][:, 0:512],
                                 start=(p == 4), stop=(p == 5))
            pb = ps.tile([128, 256], F32, name="ps2b", tag="proj", bufs=2,
                         padded_shape=[128, 512])
            for p in (4, 5):
                nc.tensor.matmul(pb, ctxt[p][:, s * 128:(s + 1) * 128],
                                 wo_t[p][:, 512:768],
                                 start=(p == 4), stop=(p == 5))
            z0 = z_p.tile([128, D], F32, name="z0_sb", bufs=2)
            nc.vector.tensor_add(out=z0[:, 0:512], in0=pa,
                                 in1=po_acc[:, s, 0:512])
            nc.vector.tensor_add(out=z0[:, 512:768], in0=pb,
                                 in1=po_acc[:, s, 512:768])
            stats = z_p.tile([128, 3, 6], F32, name="stats", bufs=2)
            for g in range(3):
                nc.vector.bn_stats(out=stats[:, g, :],
                                   in_=z0[:, g * 256:(g + 1) * 256])
            mv = z_p.tile([128, 2], F32, name="mv", bufs=2)
            nc.vector.bn_aggr(out=mv, in_=stats)
            stdv = z_p.tile([128, 1], F32, name="stdv", bufs=2)
            nc.scalar.activation(stdv, mv[:, 1:2], AF.Sqrt, bias=eps_t)
            rstd = z_p.tile([128, 1], F32, name="rstd", bufs=2)
            nc.vector.reciprocal(out=rstd, in_=stdv)
            nmr = z_p.tile([128, 1], F32, name="nmr", bufs=2)
            nc.vector.tensor_scalar(out=nmr, in0=mv[:, 0:1], scalar1=rstd,
                                    scalar2=-1.0, op0=mybir.AluOpType.mult,
                                    op1=mybir.AluOpType.mult)
            z = z_p.tile([128, D], F16, name="z_sb", bufs=2)
            nc.scalar.activation(z, z0, AF.Identity, bias=nmr, scale=rstd)
            nc.vector.tensor_mul(out=z, in0=z, in1=gamma_t)
            nc.vector.tensor_add(out=z, in0=z, in1=beta_t)
            nc.sync.dma_start(out=out_d[s * 128:(s + 1) * 128, :], in_=z)

    nc.compile()
    return nc


def _host_inputs(inputs):
    x = np.asarray(inputs["input_tensor"], np.float32)
    mask = np.asarray(inputs["attention_mask"])
    Wq = np.asarray(inputs["Wq"], np.float32)
    bq = np.asarray(inputs["bq"], np.float32)
    Wk = np.asarray(inputs["Wk"], np.float32)
    bk = np.asarray(inputs["bk"], np.float32)
    Wv = np.asarray(inputs["Wv"], np.float32)
    bv = np.asarray(inputs["bv"], np.float32)
    Wo = np.asarray(inputs["Wo"], np.float32)
    bo = np.asarray(inputs["bo"], np.float32)
    gamma = np.asarray(inputs["gamma"], np.float32)
    beta = np.asarray(inputs["beta"], np.float32)

    scale = np.float32(1.0 / np.sqrt(DH))
    wq_flat = np.ascontiguousarray(
        (Wq * scale).transpose(1, 0, 2).reshape(D, D))
    wk_flat = np.ascontiguousarray(Wk.transpose(1, 0, 2).reshape(D, D))
    bq_s = (bq * scale).reshape(D)
    bk_s = bk.reshape(D)

    wv_aug = np.zeros((D, NQUAD * 260), np.float32)
    bv_aug = np.zeros((1, NQUAD * 260), np.float32)
    for h in range(H):
        q, l = divmod(h, 4)
        base = q * 260 + l * 65
        wv_aug[:, base:base + 64] = Wv[h]
        bv_aug[0, base:base + 64] = bv[h]
        bv_aug[0, base + 64] = 1.0

    bqk = np.zeros((128, 2 * NPAIR), np.float32)
    for p in range(NPAIR):
        bqk[:, p] = bq_s[p * 128:(p + 1) * 128]
        bqk[:, NPAIR + p] = bk_s[p * 128:(p + 1) * 128]

    def sbuf_layout(w, width):
        # [D, n*width] -> [n, 128, DCH*width]: partition-major per tile
        n = w.shape[1] // width
        return np.ascontiguousarray(
            w.reshape(DCH, 128, n, width).transpose(2, 1, 0, 3).reshape(
                n, 128, DCH * width).astype(np.float16))

    shared = {
        "wq": sbuf_layout(wq_flat, 128), "wk": sbuf_layout(wk_flat, 128),
        "wv": sbuf_layout(wv_aug, 260),
        "wo": sbuf_layout(np.ascontiguousarray(Wo), D)[0],
        "bqk": bqk, "bv": bv_aug,
        "gamma16": gamma.reshape(1, D).astype(np.float16),
        "beta16": beta.reshape(1, D).astype(np.float16),
        "onesr": np.ones((1, 128), np.float32),
        "bor": bo.reshape(1, D).copy(),
        "ident": np.eye(128, dtype=np.float16),
    }
    in_maps = []
    for b in range(B):
        mb = np.where(mask[b], 0.0, NEG_MASK).astype(np.float32)
        in_maps.append({
            **shared,
            "xt": np.ascontiguousarray(
                x[b].T.reshape(DCH, 128, S).transpose(1, 0, 2).reshape(
                    128, DCH * S).astype(np.float16)),
            "maskb": np.ascontiguousarray(mb.reshape(SBLK, 128).T),
        })
    return in_maps


def _get_program():
    global _PROGRAM
    if _PROGRAM is None:
        _PROGRAM = _build_program()
    return _PROGRAM


def kernel(**inputs):
    from concourse.bass_utils import run_bass_kernel_spmd

    nc = _get_program()
    in_maps = _host_inputs(inputs)
    res = run_bass_kernel_spmd(nc, in_maps, list(range(B)))
    return np.stack(
        [res.results[b]["out"].astype(np.float32) for b in range(B)], axis=0)


if __name__ == "__main__":
    rng = np.random.default_rng(0)
    demo = {
        "input_tensor": rng.standard_normal((B, S, D)).astype(np.float32),
        "attention_mask": np.ones((B, S), bool),
        "Wq": rng.standard_normal((H, D, DH)).astype(np.float32) * 0.03,
        "bq": rng.standard_normal((H, DH)).astype(np.float32) * 0.03,
        "Wk": rng.standard_normal((H, D, DH)).astype(np.float32) * 0.03,
        "bk": rng.standard_normal((H, DH)).astype(np.float32) * 0.03,
        "Wv": rng.standard_normal((H, D, DH)).astype(np.float32) * 0.03,
        "bv": rng.standard_normal((H, DH)).astype(np.float32) * 0.03,
        "Wo": rng.standard_normal((D, D)).astype(np.float32) * 0.03,
        "bo": rng.standard_normal((D,)).astype(np.float32) * 0.03,
        "gamma": np.ones((D,), np.float32),
        "beta": np.zeros((D,), np.float32),
    }
    out = kernel(**demo)
    print("kernel ran, out shape", out.shape, "finite:", np.isfinite(out).all())
